# revision 7
# baseline (speedup 1.0000x reference)
"""Spiking-transformer attention block on 8 trn2 NeuronCores.

Reference (per problem):
    xs = spike(x); qkv = xs @ Wqkv^T; BN over (B,N); q,k,v = spike(...)
    attn = spike(q @ k^T); out = spike((attn @ v) * hd**-0.5)
    y = out @ Wproj^T + bproj
with spike(x) = floor(clip(x, 0, 4) + 0.5).

Sharding: core i handles batch b=i//2, token half h=i%2 (512 tokens).
Collectives: AllReduce of BN stats (all 8), AllGather of spiked k/v
(bf16, core pairs). All attention arithmetic is exact: spiked values
are small ints, bf16 operands + fp32 PSUM accumulation.

n_iters>1 builds a serialized K-iteration chain of the same computation
for differential timing (cancels host/dispatch overhead).
"""
import sys

if "/opt/trn_rl_repo" not in sys.path:
    sys.path.insert(0, "/opt/trn_rl_repo")

import numpy as np

B, N, C = 4, 1024, 512
H, HD = 8, 64
D3 = 3 * C            # 1536
NL = 512              # local tokens per core
NT = D3 // 128        # 12 qkv^T partition tiles
NCK = C // 128        # 4 contraction chunks
EPS = 1e-5

_CACHE = {}


def _build(n_iters=1):
    import concourse.mybir as mybir
    from concourse import bacc
    from concourse.tile import TileContext
    from concourse.masks import make_identity
    from concourse.bass import _add_dep_helper

    fp32 = mybir.dt.float32
    bf16 = mybir.dt.bfloat16
    i32 = mybir.dt.int32
    A = mybir.AluOpType
    ACTF = mybir.ActivationFunctionType

    nc = bacc.Bacc("TRN2", num_devices=8)

    # const APs used as activation bias operands
    def reg_const(val, dtype=fp32):
        t = nc.alloc_sbuf_tensor(f"const-{dtype.name}-{val}", [128, 1], dtype)
        nc.gpsimd.memset(t.ap(), val)
        nc.const_aps.aps[(dtype, val)] = t.ap()

    reg_const(4.0)
    reg_const(float(EPS))
    nc.all_engine_barrier()

    # ---- I/O ----
    xT = nc.dram_tensor("xT", [C, NL], fp32, kind="ExternalInput")
    wqkvT = nc.dram_tensor("wqkvT", [C, D3], fp32, kind="ExternalInput")
    wprojT = nc.dram_tensor("wprojT", [C, C], fp32, kind="ExternalInput")
    gamma_t = nc.dram_tensor("gamma_t", [128, NT], fp32, kind="ExternalInput")
    beta_t = nc.dram_tensor("beta_t", [128, NT], fp32, kind="ExternalInput")
    bproj_bc = nc.dram_tensor("bproj_bc", [128, C], fp32, kind="ExternalInput")
    out_ext = nc.dram_tensor("out", [NL, C], fp32, kind="ExternalOutput")

    # ---- internal DRAM (collective bounce buffers) ----
    stats_in = nc.dram_tensor("stats_in", [128, 2 * NT], fp32)
    stats_out = nc.dram_tensor("stats_out", [128, 2 * NT], fp32, addr_space="Shared")
    kv_in = nc.dram_tensor("kv_in", [2 * NL, C], bf16)
    kv_out = nc.dram_tensor("kv_out", [4 * NL, C], bf16)

    with TileContext(nc) as tc:
        with (
            tc.tile_pool(name="persist", bufs=1) as pp,
            tc.tile_pool(name="work", bufs=3) as wp,
            tc.tile_pool(name="scratch", bufs=2) as sp,
            tc.tile_pool(name="qkvps", bufs=2, space="PSUM") as qkv_ps,
            tc.tile_pool(name="vtps", bufs=1, space="PSUM") as vt_ps,
            tc.tile_pool(name="atps", bufs=3, space="PSUM") as at_ps,
            tc.tile_pool(name="avps", bufs=2, space="PSUM") as av_ps,
        ):
            # ---- identity for PE transpose ----
            ident = pp.tile([128, 128], bf16, tag="ident")
            make_identity(nc, ident[:])

            # ---- load W tiles ----
            wq_sb = []  # wqkvT [C, D3] -> 4 c-chunks x [128, D3]
            for kk in range(NCK):
                t = pp.tile([128, D3], fp32, tag=f"wq{kk}")
                nc.sync.dma_start(t[:], wqkvT[128 * kk:128 * (kk + 1), :])
                wq_sb.append(t)
            wp_sb = []
            for kk in range(NCK):
                t = pp.tile([128, C], fp32, tag=f"wp{kk}")
                nc.sync.dma_start(t[:], wprojT[128 * kk:128 * (kk + 1), :])
                wp_sb.append(t)
            gamma_sb = pp.tile([128, NT], fp32, tag="gamma")
            beta_sb = pp.tile([128, NT], fp32, tag="beta")
            bproj_sb = pp.tile([128, C], fp32, tag="bproj")
            nc.sync.dma_start(gamma_sb[:], gamma_t[:])
            nc.sync.dma_start(beta_sb[:], beta_t[:])
            nc.sync.dma_start(bproj_sb[:], bproj_bc[:])

            last_inst = None
            for it in range(n_iters):
                # ---- phase 0: x^T -> head spike -> xs^T (fp32) ----
                xs_f = []
                for kk in range(NCK):
                    raw = wp.tile([128, NL], fp32, tag="xraw")
                    dma = nc.sync.dma_start(raw[:], xT[128 * kk:128 * (kk + 1), :])
                    if last_inst is not None:
                        _add_dep_helper(dma.ins, last_inst.ins, True, "iter chain")
                    spk = wp.tile([128, NL], i32, tag="xspk")
                    nc.vector.tensor_scalar(spk[:], raw[:], 0.0, 4.0, A.max, A.min)
                    xf = pp.tile([128, NL], fp32, tag=f"xs{kk}")
                    nc.vector.tensor_copy(xf[:], spk[:])
                    xs_f.append(xf)

                # ---- phase 1: qkv^T = W^T.T @ xs^T, with stats ----
                stat_sb = pp.tile([128, 2 * NT], fp32, tag="stat")
                qkvT_sb = []
                for t in range(NT):
                    ps = qkv_ps.tile([128, NL], fp32, tag="qkvps")
                    for kk in range(NCK):
                        nc.tensor.matmul(
                            ps[:],
                            wq_sb[kk][:, 128 * t:128 * (t + 1)],
                            xs_f[kk][:],
                            start=(kk == 0),
                            stop=(kk == NCK - 1),
                        )
                    sb = pp.tile([128, NL], fp32, tag=f"qkvT{t}")
                    nc.vector.tensor_copy(sb[:], ps[:])
                    # per-channel sum and sum-of-squares on ACT (reads PSUM)
                    sum_scr = sp.tile([128, NL], fp32, tag="sumscr")
                    nc.scalar.activation(
                        sum_scr[:], ps[:], ACTF.Copy,
                        accum_out=stat_sb[:, t:t + 1],
                    )
                    sq_scr = sp.tile([128, NL], fp32, tag="sqscr")
                    nc.scalar.activation(
                        sq_scr[:], ps[:], ACTF.Square,
                        accum_out=stat_sb[:, NT + t:NT + t + 1],
                    )
                    qkvT_sb.append(sb)

                # ---- phase 2: AllReduce stats; s = gamma*rsqrt(var+eps), b = beta - mu*s
                nc.sync.dma_start(stats_in[:], stat_sb[:])
                nc.gpsimd.collective_compute(
                    "AllReduce", A.add,
                    replica_groups=[list(range(8))],
                    ins=[stats_in[:]],
                    outs=[stats_out[:]],
                )
                stat_g = pp.tile([128, 2 * NT], fp32, tag="statg")
                nc.sync.dma_start(stat_g[:], stats_out[:])
                inv_n = 1.0 / (B * N)
                mu = pp.tile([128, NT], fp32, tag="mu")
                nc.vector.tensor_scalar(mu[:], stat_g[:, 0:NT], inv_n, None, A.mult)
                ex2 = pp.tile([128, NT], fp32, tag="ex2")
                nc.vector.tensor_scalar(ex2[:], stat_g[:, NT:2 * NT], inv_n, None, A.mult)
                var = pp.tile([128, NT], fp32, tag="var")
                nc.vector.tensor_tensor(var[:], mu[:], mu[:], A.mult)
                nc.vector.tensor_tensor(var[:], ex2[:], var[:], A.subtract)
                std = pp.tile([128, NT], fp32, tag="std")
                nc.scalar.activation(std[:], var[:], ACTF.Sqrt, bias=float(EPS))
                rstd = pp.tile([128, NT], fp32, tag="rstd")
                nc.vector.reciprocal(rstd[:], std[:])
                s_all = pp.tile([128, NT], fp32, tag="sall")
                nc.vector.tensor_tensor(s_all[:], rstd[:], gamma_sb[:], A.mult)
                b_all = pp.tile([128, NT], fp32, tag="ball")
                nc.vector.tensor_tensor(b_all[:], mu[:], s_all[:], A.mult)
                nc.vector.tensor_tensor(b_all[:], beta_sb[:], b_all[:], A.subtract)

                # ---- phase 3: BN + spike -> q^T, k^T (bf16), v^T (bf16) ----
                def bn_spike(t, out_dtype=bf16):
                    bn = sp.tile([128, NL], fp32, tag="bn")
                    nc.vector.tensor_scalar(
                        bn[:], qkvT_sb[t][:],
                        s_all[:, t:t + 1], b_all[:, t:t + 1], A.mult, A.add,
                    )
                    spk = sp.tile([128, NL], i32, tag="spk")
                    nc.vector.tensor_scalar(spk[:], bn[:], 0.0, 4.0, A.max, A.min)
                    o = pp.tile([128, NL], out_dtype, tag=f"spike{t}")
                    nc.vector.tensor_copy(o[:], spk[:])
                    return o

                qT = [bn_spike(t) for t in range(0, 4)]
                kT_loc = [bn_spike(t) for t in range(4, 8)]
                vT_loc = [bn_spike(t) for t in range(8, 12)]

                # ---- phase 4: transpose local v: v_loc[j] [128 tok, 512 vch] ----
                v_loc = []
                for j in range(NCK):
                    ps = vt_ps.tile([128, C], bf16, tag="vtps")
                    for t in range(NCK):
                        nc.tensor.transpose(
                            ps[:, 128 * t:128 * (t + 1)],
                            vT_loc[t][:, 128 * j:128 * (j + 1)],
                            ident[:],
                        )
                    sb = pp.tile([128, C], bf16, tag=f"vloc{j}")
                    nc.vector.tensor_copy(sb[:], ps[:])
                    v_loc.append(sb)

                # ---- phase 5: AllGather k^T + v blocks between core pairs ----
                for t in range(NCK):
                    nc.sync.dma_start(kv_in[128 * t:128 * (t + 1), :], kT_loc[t][:])
                for j in range(NCK):
                    nc.sync.dma_start(
                        kv_in[NL + 128 * j:NL + 128 * (j + 1), :], v_loc[j][:])
                nc.gpsimd.collective_compute(
                    "AllGather", A.bypass,
                    replica_groups=[[0, 1], [2, 3], [4, 5], [6, 7]],
                    ins=[kv_in[:]],
                    outs=[kv_out[:]],
                )
                # kT_full[p]: [128 kch, 1024 m]; v_full[j]: [128 m, 512 vch]
                kT_full = []
                for p in range(NCK):
                    t = pp.tile([128, N], bf16, tag=f"ktf{p}")
                    for r in range(2):
                        nc.sync.dma_start(
                            t[:, NL * r:NL * (r + 1)],
                            kv_out[2 * NL * r + 128 * p:2 * NL * r + 128 * (p + 1), :],
                        )
                    kT_full.append(t)
                v_full = []
                for j in range(8):
                    r, jj = j // 4, j % 4
                    t = pp.tile([128, C], bf16, tag=f"vf{j}")
                    nc.sync.dma_start(
                        t[:],
                        kv_out[2 * NL * r + NL + 128 * jj:
                               2 * NL * r + NL + 128 * (jj + 1), :],
                    )
                    v_full.append(t)

                # ---- phase 6: attention per head pair ----
                sT = []      # spiked (attn@v * scale): s^T c-chunks [128, NL] fp32
                clip_idx = 0
                for p in range(4):
                    avT = av_ps.tile([128, NL], fp32, tag="avps")
                    for j in range(8):
                        # q@k^T -> attn^T chunk [128 m, 512 n], 2 heads row-tiled
                        at_A = at_ps.tile([128, NL], fp32, tag="atps")
                        at_B = at_ps.tile([128, NL], fp32, tag="atps")
                        nc.tensor.matmul(
                            at_A[:],
                            kT_full[p][0:64, 128 * j:128 * (j + 1)],
                            qT[p][0:64, :],
                            start=True, stop=True,
                            tile_position=(0, 0),
                        )
                        nc.tensor.matmul(
                            at_B[:],
                            kT_full[p][64:128, 128 * j:128 * (j + 1)],
                            qT[p][64:128, :],
                            start=True, stop=True,
                            tile_position=(64, 0),
                        )
                        # spike(attn) = min(attn, 4): split DVE / ACT
                        ab_bf = []
                        for ps_t in (at_A, at_B):
                            o = sp.tile([128, NL], bf16, tag="atbf", bufs=6)
                            if clip_idx % 8 < 5:
                                nc.vector.tensor_scalar(
                                    o[:], ps_t[:], 4.0, None, A.min)
                            else:
                                z = sp.tile([128, NL], fp32, tag="atscr")
                                nc.scalar.activation(
                                    z[:], ps_t[:], ACTF.Relu, bias=4.0, scale=-1.0)
                                nc.scalar.activation(
                                    o[:], z[:], ACTF.Relu, bias=4.0, scale=-1.0)
                            clip_idx += 1
                            ab_bf.append(o)
                        # attn@v: out^T [128 (2 hd), 512 n], 2 heads col-tiled
                        nc.tensor.matmul(
                            avT[0:64, :],
                            v_full[j][:, 128 * p:128 * p + 64],
                            ab_bf[0][:],
                            start=(j == 0), stop=(j == 7),
                            tile_position=(0, 0),
                            skip_group_check=True,
                        )
                        nc.tensor.matmul(
                            avT[64:128, :],
                            v_full[j][:, 128 * p + 64:128 * (p + 1)],
                            ab_bf[1][:],
                            start=(j == 0), stop=(j == 7),
                            tile_position=(0, 64),
                            skip_group_check=True,
                        )
                    # project_spike: s = min(floor((av+4)/8), 4) via RNE trick
                    z = sp.tile([128, NL], fp32, tag="avz")
                    nc.vector.tensor_scalar(z[:], avT[:], 0.125, 0.03125,
                                            A.mult, A.add)
                    zi = sp.tile([128, NL], i32, tag="avzi")
                    nc.vector.tensor_scalar(zi[:], z[:], 4.2, None, A.min)
                    sf = pp.tile([128, NL], fp32, tag=f"sT{p}")
                    nc.vector.tensor_copy(sf[:], zi[:])
                    sT.append(sf)

                # ---- phase 7: proj: out[nj] = s^T.T @ Wproj^T + bias ----
                for nj in range(NCK):
                    ps = qkv_ps.tile([128, C], fp32, tag="qkvps")
                    for cp in range(NCK):
                        nc.tensor.matmul(
                            ps[:],
                            sT[cp][:, 128 * nj:128 * (nj + 1)],
                            wp_sb[cp][:],
                            start=(cp == 0),
                            stop=(cp == NCK - 1),
                        )
                    ob = sp.tile([128, C], fp32, tag="outsb")
                    nc.vector.tensor_tensor(ob[:], ps[:], bproj_sb[:], A.add)
                    last_inst = nc.sync.dma_start(
                        out_ext[128 * nj:128 * (nj + 1), :], ob[:])

    nc.finalize()
    return nc


def _get_nc():
    if "nc" not in _CACHE:
        _CACHE["nc"] = _build()
    return _CACHE["nc"]


def make_in_maps(x, Wqkv, gamma, beta, Wproj, bproj):
    x = np.asarray(x, dtype=np.float32)
    wqkvT = np.ascontiguousarray(np.asarray(Wqkv, dtype=np.float32).T)
    wprojT = np.ascontiguousarray(np.asarray(Wproj, dtype=np.float32).T)
    gamma_t = np.ascontiguousarray(np.asarray(gamma, np.float32).reshape(NT, 128).T)
    beta_t = np.ascontiguousarray(np.asarray(beta, np.float32).reshape(NT, 128).T)
    bproj_b = np.ascontiguousarray(
        np.broadcast_to(np.asarray(bproj, np.float32), (128, C)))
    in_maps = []
    for i in range(8):
        b, h = i // 2, i % 2
        xTl = np.ascontiguousarray(x[b, h * NL:(h + 1) * NL, :].T)
        in_maps.append({
            "xT": xTl,
            "wqkvT": wqkvT,
            "wprojT": wprojT,
            "gamma_t": gamma_t,
            "beta_t": beta_t,
            "bproj_bc": bproj_b,
        })
    return in_maps


def kernel(x, Wqkv, gamma, beta, Wproj, bproj, **_ignored):
    from concourse.bass_utils import run_bass_kernel_spmd

    nc = _get_nc()
    in_maps = make_in_maps(x, Wqkv, gamma, beta, Wproj, bproj)
    res = run_bass_kernel_spmd(nc, in_maps, core_ids=list(range(8)))
    out = np.empty((B, N, C), np.float32)
    for i in range(8):
        b, h = i // 2, i % 2
        out[b, h * NL:(h + 1) * NL, :] = res.results[i]["out"]
    return out


# revision 13
# speedup vs baseline: 1.9536x; 1.9536x over previous
"""Spiking-transformer attention block on 8 trn2 NeuronCores.

Reference (per problem):
    xs = spike(x); qkv = xs @ Wqkv^T; BN over (B,N); q,k,v = spike(...)
    attn = spike(q @ k^T); out = spike((attn @ v) * hd**-0.5)
    y = out @ Wproj^T + bproj
with spike(x) = floor(clip(x, 0, 4) + 0.5).

Sharding: core i handles batch b=i//2, token half h=i%2 (512 tokens).
Collectives: AllReduce of BN stats (all 8), AllGather of spiked k/v
(bf16, core pairs). All attention arithmetic is exact: spiked values
are small ints, bf16 operands + fp32 PSUM accumulation.

n_iters>1 builds a serialized K-iteration chain of the same computation
for differential timing (cancels host/dispatch overhead).
"""
import sys

if "/opt/trn_rl_repo" not in sys.path:
    sys.path.insert(0, "/opt/trn_rl_repo")

import numpy as np

B, N, C = 4, 1024, 512
H, HD = 8, 64
D3 = 3 * C            # 1536
NL = 512              # local tokens per core
NT = D3 // 128        # 12 qkv^T partition tiles
NCK = C // 128        # 4 contraction chunks
EPS = 1e-5

_CACHE = {}


def _build(n_iters=1, stub_coll=False, stub_attn=False, clip_act_mod=3):
    import concourse.mybir as mybir
    from concourse import bacc
    from concourse.tile import TileContext
    from concourse.bass import _add_dep_helper

    fp32 = mybir.dt.float32
    bf16 = mybir.dt.bfloat16
    i32 = mybir.dt.int32
    A = mybir.AluOpType
    ACTF = mybir.ActivationFunctionType

    nc = bacc.Bacc("TRN2", num_devices=8)

    # const APs used as activation bias operands
    def reg_const(val, dtype=fp32):
        t = nc.alloc_sbuf_tensor(f"const-{dtype.name}-{val}", [128, 1], dtype)
        nc.gpsimd.memset(t.ap(), val)
        nc.const_aps.aps[(dtype, val)] = t.ap()

    reg_const(4.0)
    reg_const(0.03125)
    reg_const(float(EPS))
    nc.all_engine_barrier()

    # ---- I/O ----
    xT = nc.dram_tensor("xT", [C, NL], fp32, kind="ExternalInput")
    wqkvT = nc.dram_tensor("wqkvT", [C, D3], fp32, kind="ExternalInput")
    wprojT = nc.dram_tensor("wprojT", [C, C], fp32, kind="ExternalInput")
    gamma_t = nc.dram_tensor("gamma_t", [128, NT], fp32, kind="ExternalInput")
    beta_t = nc.dram_tensor("beta_t", [128, NT], fp32, kind="ExternalInput")
    bproj_bc = nc.dram_tensor("bproj_bc", [128, C], fp32, kind="ExternalInput")
    out_ext = nc.dram_tensor("out", [NL, C], fp32, kind="ExternalOutput")

    # ---- internal DRAM (collective bounce buffers) ----
    stats_in = nc.dram_tensor("stats_in", [128, 2 * NT], fp32)
    stats_out = nc.dram_tensor(
        "stats_out", [128, 2 * NT], fp32,
        addr_space="Local" if stub_coll else "Shared")
    kv_in = nc.dram_tensor("kv_in", [2 * NL, C], bf16)
    kv_out = nc.dram_tensor("kv_out", [4 * NL, C], bf16)

    with TileContext(nc) as tc:
        with (
            tc.tile_pool(name="persist", bufs=1) as pp,
            tc.tile_pool(name="work", bufs=3) as wp,
            tc.tile_pool(name="scratch", bufs=2) as sp,
        ):
            # ---- load W tiles ----
            wq_sb = []  # wqkvT [C, D3] -> 4 c-chunks x [128, D3]
            for kk in range(NCK):
                t = pp.tile([128, D3], fp32, tag=f"wq{kk}", name=f"wq{kk}")
                nc.sync.dma_start(t[:], wqkvT[128 * kk:128 * (kk + 1), :])
                wq_sb.append(t)
            wp_sb = []
            for kk in range(NCK):
                t = pp.tile([128, C], fp32, tag=f"wp{kk}", name=f"wp{kk}")
                nc.sync.dma_start(t[:], wprojT[128 * kk:128 * (kk + 1), :])
                wp_sb.append(t)
            gamma_sb = pp.tile([128, NT], fp32, tag="gamma")
            beta_sb = pp.tile([128, NT], fp32, tag="beta")
            bproj_sb = pp.tile([128, C], fp32, tag="bproj")
            nc.sync.dma_start(gamma_sb[:], gamma_t[:])
            nc.sync.dma_start(beta_sb[:], beta_t[:])
            nc.sync.dma_start(bproj_sb[:], bproj_bc[:])

            last_inst = None
            for it in range(n_iters):
                # ---- phase 0: x^T -> head spike -> xs^T (fp32) ----
                xs_f = []
                for kk in range(NCK):
                    raw = wp.tile([128, NL], fp32, tag="xraw", name="xraw")
                    dma = nc.sync.dma_start(raw[:], xT[128 * kk:128 * (kk + 1), :])
                    if last_inst is not None:
                        _add_dep_helper(dma.ins, last_inst.ins, True, "iter chain")
                    spk = wp.tile([128, NL], i32, tag="xspk", name="xspk")
                    nc.vector.tensor_scalar(spk[:], raw[:], 0.0, 4.0, A.max, A.min)
                    xf = pp.tile([128, NL], fp32, tag=f"xs{kk}", name=f"xs{kk}")
                    nc.vector.tensor_copy(xf[:], spk[:])
                    xs_f.append(xf)

                # ---- phase 1: qkv^T = W^T.T @ xs^T, with stats ----
                stat_sb = pp.tile([128, 2 * NT], fp32, tag="stat", name="stat")
                qkvT_sb = []
                with tc.tile_pool(name="qkvps", bufs=3, space="PSUM") as qkv_ps:
                    for t in range(NT):
                        ps = qkv_ps.tile([128, NL], fp32, tag="qkvps", name="qkvps")
                        for kk in range(NCK):
                            nc.tensor.matmul(
                                ps[:],
                                wq_sb[kk][:, 128 * t:128 * (t + 1)],
                                xs_f[kk][:],
                                start=(kk == 0),
                                stop=(kk == NCK - 1),
                            )
                        sb = pp.tile([128, NL], fp32, tag=f"qkvT{t}", name=f"qkvT{t}")
                        nc.vector.tensor_copy(sb[:], ps[:])
                        # per-channel sum / sum-of-squares on ACT (reads PSUM)
                        sum_scr = sp.tile([128, NL], fp32, tag="sumscr", name="sumscr")
                        nc.scalar.activation(
                            sum_scr[:], ps[:], ACTF.Copy,
                            accum_out=stat_sb[:, t:t + 1],
                        )
                        sq_scr = sp.tile([128, NL], fp32, tag="sqscr", name="sqscr")
                        nc.scalar.activation(
                            sq_scr[:], ps[:], ACTF.Square,
                            accum_out=stat_sb[:, NT + t:NT + t + 1],
                        )
                        qkvT_sb.append(sb)

                # ---- phase 2: AllReduce stats; s = gamma*rsqrt(var+eps), b = beta-mu*s
                nc.sync.dma_start(stats_in[:], stat_sb[:])
                if stub_coll:
                    nc.sync.dma_start(stats_out[:], stats_in[:])
                else:
                    nc.gpsimd.collective_compute(
                        "AllReduce", A.add,
                        replica_groups=[list(range(8))],
                        ins=[stats_in[:]],
                        outs=[stats_out[:]],
                    )
                stat_g = pp.tile([128, 2 * NT], fp32, tag="statg", name="statg")
                nc.sync.dma_start(stat_g[:], stats_out[:])
                inv_n = 1.0 / (B * N)
                mu = pp.tile([128, NT], fp32, tag="mu", name="mu")
                nc.vector.tensor_scalar(mu[:], stat_g[:, 0:NT], inv_n, None, A.mult)
                ex2 = pp.tile([128, NT], fp32, tag="ex2", name="ex2")
                nc.vector.tensor_scalar(ex2[:], stat_g[:, NT:2 * NT], inv_n,
                                        None, A.mult)
                var = pp.tile([128, NT], fp32, tag="var", name="var")
                nc.vector.tensor_tensor(var[:], mu[:], mu[:], A.mult)
                nc.vector.tensor_tensor(var[:], ex2[:], var[:], A.subtract)
                std = pp.tile([128, NT], fp32, tag="std", name="std")
                nc.scalar.activation(std[:], var[:], ACTF.Sqrt, bias=float(EPS))
                rstd = pp.tile([128, NT], fp32, tag="rstd", name="rstd")
                nc.vector.reciprocal(rstd[:], std[:])
                s_all = pp.tile([128, NT], fp32, tag="sall", name="sall")
                nc.vector.tensor_tensor(s_all[:], rstd[:], gamma_sb[:], A.mult)
                b_all = pp.tile([128, NT], fp32, tag="ball", name="ball")
                nc.vector.tensor_tensor(b_all[:], mu[:], s_all[:], A.mult)
                nc.vector.tensor_tensor(b_all[:], beta_sb[:], b_all[:], A.subtract)

                # ---- phase 3: BN + spike -> q^T, k^T, v^T (bf16) ----
                def bn_spike(t):
                    # max(0, s*x+b) on ACT; min(.,4)->int32 (RNE); ->bf16
                    bn = sp.tile([128, NL], fp32, tag="bn", name="bn")
                    nc.scalar.activation(
                        bn[:], qkvT_sb[t][:], ACTF.Relu,
                        bias=b_all[:, t:t + 1], scale=s_all[:, t:t + 1])
                    spk = sp.tile([128, NL], i32, tag="spk", name="spk")
                    nc.vector.tensor_scalar(spk[:], bn[:], 4.0, None, A.min)
                    o = pp.tile([128, NL], bf16, tag=f"spike{t}", name=f"spike{t}")
                    nc.vector.tensor_copy(o[:], spk[:])
                    return o

                qT = [bn_spike(t) for t in range(0, 4)]
                kT_loc = [bn_spike(t) for t in range(4, 8)]
                vT_loc = [bn_spike(t) for t in range(8, 12)]

                # ---- phase 4: transpose local v via DMA-transpose ----
                # v_loc[j] [128 tok, 512 vch] <- vT_loc[t][:, 128j:128(j+1)].T
                v_loc = []
                for j in range(NCK):
                    sb = pp.tile([128, C], bf16, tag=f"vloc{j}", name=f"vloc{j}")
                    for t in range(NCK):
                        nc.sync.dma_start(
                            sb[:, 128 * t:128 * (t + 1)],
                            vT_loc[t][:, 128 * j:128 * (j + 1)],
                            transpose=True)
                    v_loc.append(sb)

                # ---- phase 5: AllGather k^T + v blocks between core pairs ----
                for t in range(NCK):
                    nc.sync.dma_start(kv_in[128 * t:128 * (t + 1), :], kT_loc[t][:])
                for j in range(NCK):
                    nc.sync.dma_start(
                        kv_in[NL + 128 * j:NL + 128 * (j + 1), :], v_loc[j][:])
                if stub_coll:
                    nc.sync.dma_start(kv_out[0:2 * NL, :], kv_in[:])
                    nc.sync.dma_start(kv_out[2 * NL:4 * NL, :], kv_in[:])
                else:
                    nc.gpsimd.collective_compute(
                        "AllGather", A.bypass,
                        replica_groups=[[0, 1], [2, 3], [4, 5], [6, 7]],
                        ins=[kv_in[:]],
                        outs=[kv_out[:]],
                    )
                # kT_full[p]: [128 kch, 1024 m]; v_full[j]: [128 m, 512 vch]
                kT_full = []
                for p in range(NCK):
                    t = pp.tile([128, N], bf16, tag=f"ktf{p}", name=f"ktf{p}")
                    for r in range(2):
                        nc.sync.dma_start(
                            t[:, NL * r:NL * (r + 1)],
                            kv_out[2 * NL * r + 128 * p:2 * NL * r + 128 * (p + 1), :],
                        )
                    kT_full.append(t)
                v_full = []
                for j in range(8):
                    r, jj = j // 4, j % 4
                    t = pp.tile([128, C], bf16, tag=f"vf{j}", name=f"vf{j}")
                    nc.sync.dma_start(
                        t[:],
                        kv_out[2 * NL * r + NL + 128 * jj:
                               2 * NL * r + NL + 128 * (jj + 1), :],
                    )
                    v_full.append(t)

                # ---- phase 6: attention per head pair ----
                sT = []      # spiked (attn@v * scale): s^T c-chunks [128, NL] fp32
                with (
                    tc.tile_pool(name="atps", bufs=3, space="PSUM") as at_ps,
                    tc.tile_pool(name="avps", bufs=2, space="PSUM") as av_ps,
                ):
                    for p in range(4):
                        avT = av_ps.tile([128, NL], fp32, tag="avps", name="avps")
                        if stub_attn:
                            nc.tensor.matmul(
                                avT[:], v_full[0][:, 0:128], qT[p][:],
                                start=True, stop=True)
                        for j in range(8 if not stub_attn else 0):
                            # q@k^T -> attn^T [128 m, 1024] both heads (2 banks)
                            at_AB = at_ps.tile([128, 2 * NL], fp32, tag="atps",
                                               name="atps")
                            nc.tensor.matmul(
                                at_AB[:, 0:NL],
                                kT_full[p][0:64, 128 * j:128 * (j + 1)],
                                qT[p][0:64, :],
                                start=True, stop=True,
                                tile_position=(0, 0),
                            )
                            nc.tensor.matmul(
                                at_AB[:, NL:2 * NL],
                                kT_full[p][64:128, 128 * j:128 * (j + 1)],
                                qT[p][64:128, :],
                                start=True, stop=True,
                                tile_position=(64, 0),
                            )
                            # spike(attn) = min(attn, 4), one op over both heads
                            ab = sp.tile([128, 2 * NL], bf16, tag="atbf",
                                         name="atbf", bufs=6)
                            if (p * 8 + j) % clip_act_mod == clip_act_mod - 1:
                                z = sp.tile([128, 2 * NL], fp32, tag="atscr",
                                            name="atscr")
                                nc.scalar.activation(
                                    z[:], at_AB[:], ACTF.Relu, bias=4.0, scale=-1.0)
                                nc.scalar.activation(
                                    ab[:], z[:], ACTF.Relu, bias=4.0, scale=-1.0)
                            else:
                                nc.vector.tensor_scalar(
                                    ab[:], at_AB[:], 4.0, None, A.min)
                            # attn@v: out^T [128 (2 hd), 512 n], heads col-tiled
                            nc.tensor.matmul(
                                avT[0:64, :],
                                v_full[j][:, 128 * p:128 * p + 64],
                                ab[:, 0:NL],
                                start=(j == 0), stop=(j == 7),
                                tile_position=(0, 0),
                                skip_group_check=True,
                            )
                            nc.tensor.matmul(
                                avT[64:128, :],
                                v_full[j][:, 128 * p + 64:128 * (p + 1)],
                                ab[:, NL:2 * NL],
                                start=(j == 0), stop=(j == 7),
                                tile_position=(0, 64),
                                skip_group_check=True,
                            )
                        # project_spike: s = min(floor((av+4)/8), 4) via RNE trick
                        z = sp.tile([128, NL], fp32, tag="avz", name="avz")
                        nc.scalar.activation(z[:], avT[:], ACTF.Relu,
                                             bias=0.03125, scale=0.125)
                        zi = sp.tile([128, NL], i32, tag="avzi", name="avzi")
                        nc.vector.tensor_scalar(zi[:], z[:], 4.2, None, A.min)
                        sf = pp.tile([128, NL], fp32, tag=f"sT{p}", name=f"sT{p}")
                        nc.vector.tensor_copy(sf[:], zi[:])
                        sT.append(sf)

                # ---- phase 7: proj: out[nj] = s^T.T @ Wproj^T + bias ----
                with tc.tile_pool(name="prps", bufs=2, space="PSUM") as pr_ps:
                    for nj in range(NCK):
                        ps = pr_ps.tile([128, C], fp32, tag="prps", name="prps")
                        for cp in range(NCK):
                            nc.tensor.matmul(
                                ps[:],
                                sT[cp][:, 128 * nj:128 * (nj + 1)],
                                wp_sb[cp][:],
                                start=(cp == 0),
                                stop=(cp == NCK - 1),
                            )
                        ob = sp.tile([128, C], fp32, tag="outsb", name="outsb")
                        nc.vector.tensor_tensor(ob[:], ps[:], bproj_sb[:], A.add)
                        last_inst = nc.sync.dma_start(
                            out_ext[128 * nj:128 * (nj + 1), :], ob[:])

    nc.finalize()
    return nc


def _get_nc():
    if "nc" not in _CACHE:
        _CACHE["nc"] = _build()
    return _CACHE["nc"]


def make_in_maps(x, Wqkv, gamma, beta, Wproj, bproj):
    x = np.asarray(x, dtype=np.float32)
    wqkvT = np.ascontiguousarray(np.asarray(Wqkv, dtype=np.float32).T)
    wprojT = np.ascontiguousarray(np.asarray(Wproj, dtype=np.float32).T)
    gamma_t = np.ascontiguousarray(np.asarray(gamma, np.float32).reshape(NT, 128).T)
    beta_t = np.ascontiguousarray(np.asarray(beta, np.float32).reshape(NT, 128).T)
    bproj_b = np.ascontiguousarray(
        np.broadcast_to(np.asarray(bproj, np.float32), (128, C)))
    in_maps = []
    for i in range(8):
        b, h = i // 2, i % 2
        xTl = np.ascontiguousarray(x[b, h * NL:(h + 1) * NL, :].T)
        in_maps.append({
            "xT": xTl,
            "wqkvT": wqkvT,
            "wprojT": wprojT,
            "gamma_t": gamma_t,
            "beta_t": beta_t,
            "bproj_bc": bproj_b,
        })
    return in_maps


def kernel(x, Wqkv, gamma, beta, Wproj, bproj, **_ignored):
    from concourse.bass_utils import run_bass_kernel_spmd

    nc = _get_nc()
    in_maps = make_in_maps(x, Wqkv, gamma, beta, Wproj, bproj)
    res = run_bass_kernel_spmd(nc, in_maps, core_ids=list(range(8)))
    out = np.empty((B, N, C), np.float32)
    for i in range(8):
        b, h = i // 2, i % 2
        out[b, h * NL:(h + 1) * NL, :] = res.results[i]["out"]
    return out


# revision 16
# speedup vs baseline: 2.1074x; 1.0787x over previous
"""Spiking-transformer attention block on 8 trn2 NeuronCores.

Reference (per problem):
    xs = spike(x); qkv = xs @ Wqkv^T; BN over (B,N); q,k,v = spike(...)
    attn = spike(q @ k^T); out = spike((attn @ v) * hd**-0.5)
    y = out @ Wproj^T + bproj
with spike(x) = floor(clip(x, 0, 4) + 0.5).

Sharding (v3): core i owns batch b=i//2, token half h=i%2. The host hands
each core its FULL batch x^T with columns rotated own-half-first, so k/v
for all 1024 tokens are computed locally (no kv AllGather; k/v work is
duplicated across the pair, which is cheap with fp32r matmuls). q and the
attention output cover only the core's own 512 tokens. The only
collective is the 12KB BatchNorm-stats AllReduce (each core contributes
stats over its own half, so every token is counted exactly once).

All attention arithmetic is exact: spiked values are small ints, bf16
operands, fp32 PSUM accumulation. Weight matmuls use float32r (full fp32
words through the fast PE path; exact in this runtime).

n_iters>1 builds a serialized K-iteration chain for differential timing.
"""
import sys

if "/opt/trn_rl_repo" not in sys.path:
    sys.path.insert(0, "/opt/trn_rl_repo")

import numpy as np

B, N, C = 4, 1024, 512
H, HD = 8, 64
D3 = 3 * C            # 1536
NL = 512              # own tokens per core
NT = D3 // 128        # 12 qkv^T partition tiles
NCK = C // 128        # 4 contraction chunks
EPS = 1e-5

_CACHE = {}


def _build(n_iters=1, stub_coll=False, stub_attn=False):
    import concourse.mybir as mybir
    from concourse import bacc
    from concourse.tile import TileContext
    from concourse.bass import _add_dep_helper

    fp32 = mybir.dt.float32
    f32r = mybir.dt.float32r
    bf16 = mybir.dt.bfloat16
    i32 = mybir.dt.int32
    A = mybir.AluOpType
    ACTF = mybir.ActivationFunctionType

    nc = bacc.Bacc("TRN2", num_devices=8)

    def reg_const(val, dtype=fp32):
        t = nc.alloc_sbuf_tensor(f"const-{dtype.name}-{val}", [128, 1], dtype)
        nc.gpsimd.memset(t.ap(), val)
        nc.const_aps.aps[(dtype, val)] = t.ap()

    reg_const(4.0)
    reg_const(0.03125)
    reg_const(float(EPS))
    nc.all_engine_barrier()

    # ---- I/O ----
    xT = nc.dram_tensor("xT", [C, N], fp32, kind="ExternalInput")
    wqkvT = nc.dram_tensor("wqkvT", [C, D3], f32r, kind="ExternalInput")
    wprojT = nc.dram_tensor("wprojT", [C, C], f32r, kind="ExternalInput")
    gamma_t = nc.dram_tensor("gamma_t", [128, NT], fp32, kind="ExternalInput")
    beta_t = nc.dram_tensor("beta_t", [128, NT], fp32, kind="ExternalInput")
    bproj_bc = nc.dram_tensor("bproj_bc", [128, C], fp32, kind="ExternalInput")
    out_ext = nc.dram_tensor("out", [NL, C], fp32, kind="ExternalOutput")

    stats_in = nc.dram_tensor("stats_in", [128, 2 * NT], fp32)
    stats_out = nc.dram_tensor(
        "stats_out", [128, 2 * NT], fp32,
        addr_space="Local" if stub_coll else "Shared")

    with TileContext(nc) as tc:
        with (
            tc.tile_pool(name="persist", bufs=1) as pp,
            tc.tile_pool(name="work", bufs=3) as wp,
            tc.tile_pool(name="scratch", bufs=2) as sp,
        ):
            # ---- load inputs (x first: first matmul waits on it) ----
            x_raw = []
            for kk in range(NCK):
                t = wp.tile([128, N], fp32, tag="xraw", name="xraw", bufs=4)
                x_raw.append(t)
            gamma_sb = pp.tile([128, NT], fp32, tag="gamma", name="gamma")
            beta_sb = pp.tile([128, NT], fp32, tag="beta", name="beta")
            bproj_sb = pp.tile([128, C], fp32, tag="bproj", name="bproj")
            wq_sb = []
            wp_sb = []
            for kk in range(NCK):
                t = pp.tile([128, D3], f32r, tag=f"wq{kk}", name=f"wq{kk}")
                wq_sb.append(t)
                t = pp.tile([128, C], f32r, tag=f"wp{kk}", name=f"wp{kk}")
                wp_sb.append(t)

            first_load = True

            def load_persistent():
                for kk in range(NCK):
                    nc.sync.dma_start(wq_sb[kk][:],
                                      wqkvT[128 * kk:128 * (kk + 1), :])
                for kk in range(NCK):
                    nc.sync.dma_start(wp_sb[kk][:],
                                      wprojT[128 * kk:128 * (kk + 1), :])
                nc.sync.dma_start(gamma_sb[:], gamma_t[:])
                nc.sync.dma_start(beta_sb[:], beta_t[:])
                nc.sync.dma_start(bproj_sb[:], bproj_bc[:])

            last_inst = None
            for it in range(n_iters):
                # ---- phase 0: x^T (full batch, own-half-first) -> spike ----
                xs_f = []
                for kk in range(NCK):
                    raw = x_raw[kk]
                    dma = nc.sync.dma_start(raw[:], xT[128 * kk:128 * (kk + 1), :])
                    if last_inst is not None:
                        _add_dep_helper(dma.ins, last_inst.ins, True, "iter chain")
                    if first_load and kk == 0:
                        load_persistent()
                    spk = wp.tile([128, N], i32, tag="xspk", name="xspk")
                    nc.gpsimd.tensor_scalar(spk[:], raw[:], 0.0, 4.0, A.max, A.min)
                    xf = pp.tile([128, N], f32r, tag=f"xs{kk}", name=f"xs{kk}")
                    nc.gpsimd.tensor_copy(xf[:], spk[:])
                    xs_f.append(xf)
                first_load = False

                # ---- phase 1: qkv^T (fp32r), stats over own half ----
                stat_sb = pp.tile([128, 2 * NT], fp32, tag="stat", name="stat")
                qkvT_sb = []
                with (
                    tc.tile_pool(name="qps", bufs=2, space="PSUM") as q_ps,
                    tc.tile_pool(name="kvps", bufs=3, space="PSUM") as kv_ps,
                ):
                    for t in range(NT):
                        is_q = t < 4
                        ncols = NL if is_q else N
                        pool = q_ps if is_q else kv_ps
                        ps = pool.tile([128, ncols], fp32,
                                       tag="qps" if is_q else "kvps",
                                       name="ps")
                        for cchunk in range(ncols // NL):
                            for kk in range(NCK):
                                nc.tensor.matmul(
                                    ps[:, NL * cchunk:NL * (cchunk + 1)],
                                    wq_sb[kk][:, 128 * t:128 * (t + 1)],
                                    xs_f[kk][:, NL * cchunk:NL * (cchunk + 1)],
                                    start=(kk == 0),
                                    stop=(kk == NCK - 1),
                                )
                        sb = pp.tile([128, ncols], fp32, tag=f"qkvT{t}",
                                     name=f"qkvT{t}")
                        # own half: evac + sum and sumsq on ACT
                        nc.scalar.activation(
                            sb[:, 0:NL], ps[:, 0:NL], ACTF.Copy,
                            accum_out=stat_sb[:, t:t + 1],
                        )
                        sq_scr = sp.tile([128, NL], fp32, tag="sqscr",
                                         name="sqscr")
                        nc.scalar.activation(
                            sq_scr[:], ps[:, 0:NL], ACTF.Square,
                            accum_out=stat_sb[:, NT + t:NT + t + 1],
                        )
                        if not is_q:
                            nc.vector.tensor_copy(sb[:, NL:N], ps[:, NL:N])
                        qkvT_sb.append(sb)

                # ---- phase 2: AllReduce stats -> s, b per channel ----
                nc.sync.dma_start(stats_in[:], stat_sb[:])
                if stub_coll:
                    nc.sync.dma_start(stats_out[:], stats_in[:])
                else:
                    nc.gpsimd.collective_compute(
                        "AllReduce", A.add,
                        replica_groups=[list(range(8))],
                        ins=[stats_in[:]],
                        outs=[stats_out[:]],
                    )
                stat_g = pp.tile([128, 2 * NT], fp32, tag="statg", name="statg")
                nc.sync.dma_start(stat_g[:], stats_out[:])
                inv_n = 1.0 / (B * N)
                mu = pp.tile([128, NT], fp32, tag="mu", name="mu")
                nc.vector.tensor_scalar(mu[:], stat_g[:, 0:NT], inv_n, None, A.mult)
                ex2 = pp.tile([128, NT], fp32, tag="ex2", name="ex2")
                nc.vector.tensor_scalar(ex2[:], stat_g[:, NT:2 * NT], inv_n,
                                        None, A.mult)
                var = pp.tile([128, NT], fp32, tag="var", name="var")
                nc.vector.tensor_tensor(var[:], mu[:], mu[:], A.mult)
                nc.vector.tensor_tensor(var[:], ex2[:], var[:], A.subtract)
                std = pp.tile([128, NT], fp32, tag="std", name="std")
                nc.scalar.activation(std[:], var[:], ACTF.Sqrt, bias=float(EPS))
                rstd = pp.tile([128, NT], fp32, tag="rstd", name="rstd")
                nc.vector.reciprocal(rstd[:], std[:])
                s_all = pp.tile([128, NT], fp32, tag="sall", name="sall")
                nc.vector.tensor_tensor(s_all[:], rstd[:], gamma_sb[:], A.mult)
                b_all = pp.tile([128, NT], fp32, tag="ball", name="ball")
                nc.vector.tensor_tensor(b_all[:], mu[:], s_all[:], A.mult)
                nc.vector.tensor_tensor(b_all[:], beta_sb[:], b_all[:], A.subtract)

                # ---- phase 3: BN + spike -> q^T, k^T, v^T (bf16) ----
                def bn_spike(t):
                    ncols = NL if t < 4 else N
                    bn = sp.tile([128, N], fp32, tag="bn", name="bn")
                    if t < 4:
                        # q tiles: all DVE
                        nc.vector.tensor_scalar(
                            bn[:, 0:ncols], qkvT_sb[t][:],
                            s_all[:, t:t + 1], b_all[:, t:t + 1],
                            A.mult, A.add)
                    else:
                        # k/v tiles: BN+relu on ACT
                        nc.scalar.activation(
                            bn[:, 0:ncols], qkvT_sb[t][:], ACTF.Relu,
                            bias=b_all[:, t:t + 1], scale=s_all[:, t:t + 1])
                    spk = sp.tile([128, N], i32, tag="spk", name="spk")
                    nc.vector.tensor_scalar(spk[:, 0:ncols], bn[:, 0:ncols],
                                            0.0, 4.0, A.max, A.min)
                    o = pp.tile([128, ncols], bf16, tag=f"spike{t}",
                                name=f"spike{t}")
                    nc.vector.tensor_copy(o[:], spk[:, 0:ncols])
                    return o

                qT = [bn_spike(t) for t in range(0, 4)]
                kT = [bn_spike(t) for t in range(4, 8)]
                vT = [bn_spike(t) for t in range(8, 12)]

                # ---- phase 4: v transpose via DMA: v_loc[j] [128 m, 512 vch]
                v_loc = []
                for j in range(8):
                    sb = pp.tile([128, C], bf16, tag=f"vloc{j}", name=f"vloc{j}")
                    for t in range(NCK):
                        nc.sync.dma_start(
                            sb[:, 128 * t:128 * (t + 1)],
                            vT[t][:, 128 * j:128 * (j + 1)],
                            transpose=True)
                    v_loc.append(sb)

                # ---- phase 6: attention per head pair ----
                sT = []
                clip_idx = 0
                with (
                    tc.tile_pool(name="atps", bufs=3, space="PSUM") as at_ps,
                    tc.tile_pool(name="avps", bufs=2, space="PSUM") as av_ps,
                ):
                    for p in range(4):
                        avT = av_ps.tile([128, NL], fp32, tag="avps", name="avps")
                        if stub_attn:
                            nc.tensor.matmul(
                                avT[:], v_loc[0][:, 0:128].bitcast(bf16),
                                qT[p][:], start=True, stop=True)
                        for j in range(8 if not stub_attn else 0):
                            at_AB = at_ps.tile([128, 2 * NL], fp32, tag="atps",
                                               name="atps")
                            nc.tensor.matmul(
                                at_AB[:, 0:NL],
                                kT[p][0:64, 128 * j:128 * (j + 1)],
                                qT[p][0:64, :],
                                start=True, stop=True,
                                tile_position=(0, 0),
                            )
                            nc.tensor.matmul(
                                at_AB[:, NL:2 * NL],
                                kT[p][64:128, 128 * j:128 * (j + 1)],
                                qT[p][64:128, :],
                                start=True, stop=True,
                                tile_position=(64, 0),
                            )
                            ab = sp.tile([128, 2 * NL], bf16, tag="atbf",
                                         name="atbf", bufs=6)
                            if clip_idx % 4 == 3:
                                z = sp.tile([128, 2 * NL], fp32, tag="atscr",
                                            name="atscr")
                                nc.scalar.activation(
                                    z[:], at_AB[:], ACTF.Relu, bias=4.0,
                                    scale=-1.0)
                                nc.scalar.activation(
                                    ab[:], z[:], ACTF.Relu, bias=4.0,
                                    scale=-1.0)
                            else:
                                nc.vector.tensor_scalar(
                                    ab[:], at_AB[:], 4.0, None, A.min)
                            clip_idx += 1
                            nc.tensor.matmul(
                                avT[0:64, :],
                                v_loc[j][:, 128 * p:128 * p + 64],
                                ab[:, 0:NL],
                                start=(j == 0), stop=(j == 7),
                                tile_position=(0, 0),
                                skip_group_check=True,
                            )
                            nc.tensor.matmul(
                                avT[64:128, :],
                                v_loc[j][:, 128 * p + 64:128 * (p + 1)],
                                ab[:, NL:2 * NL],
                                start=(j == 0), stop=(j == 7),
                                tile_position=(0, 64),
                                skip_group_check=True,
                            )
                        # project_spike: s = min(floor((av+4)/8), 4), RNE trick
                        z = sp.tile([128, NL], fp32, tag="avz", name="avz")
                        nc.scalar.activation(z[:], avT[:], ACTF.Relu,
                                             bias=0.03125, scale=0.125)
                        zi = sp.tile([128, NL], i32, tag="avzi", name="avzi")
                        nc.gpsimd.tensor_scalar(zi[:], z[:], 4.2, None, A.min)
                        sf = pp.tile([128, NL], f32r, tag=f"sT{p}", name=f"sT{p}")
                        nc.gpsimd.tensor_copy(sf[:], zi[:])
                        sT.append(sf)

                # ---- phase 7: proj (fp32r) + bias ----
                with tc.tile_pool(name="prps", bufs=2, space="PSUM") as pr_ps:
                    for nj in range(NCK):
                        ps = pr_ps.tile([128, C], fp32, tag="prps", name="prps")
                        for cp in range(NCK):
                            nc.tensor.matmul(
                                ps[:],
                                sT[cp][:, 128 * nj:128 * (nj + 1)],
                                wp_sb[cp][:],
                                start=(cp == 0),
                                stop=(cp == NCK - 1),
                            )
                        ob = sp.tile([128, C], fp32, tag="outsb", name="outsb")
                        nc.vector.tensor_tensor(ob[:], ps[:], bproj_sb[:], A.add)
                        last_inst = nc.sync.dma_start(
                            out_ext[128 * nj:128 * (nj + 1), :], ob[:])

    nc.finalize()
    return nc


def _get_nc():
    if "nc" not in _CACHE:
        _CACHE["nc"] = _build()
    return _CACHE["nc"]


def make_in_maps(x, Wqkv, gamma, beta, Wproj, bproj):
    x = np.asarray(x, dtype=np.float32)
    wqkvT = np.ascontiguousarray(np.asarray(Wqkv, dtype=np.float32).T)
    wprojT = np.ascontiguousarray(np.asarray(Wproj, dtype=np.float32).T)
    gamma_t = np.ascontiguousarray(np.asarray(gamma, np.float32).reshape(NT, 128).T)
    beta_t = np.ascontiguousarray(np.asarray(beta, np.float32).reshape(NT, 128).T)
    bproj_b = np.ascontiguousarray(
        np.broadcast_to(np.asarray(bproj, np.float32), (128, C)))
    in_maps = []
    for i in range(8):
        b, h = i // 2, i % 2
        own = x[b, h * NL:(h + 1) * NL, :].T          # [C, 512]
        other = x[b, (1 - h) * NL:(2 - h) * NL, :].T  # [C, 512]
        xTl = np.ascontiguousarray(np.concatenate([own, other], axis=1))
        in_maps.append({
            "xT": xTl,
            "wqkvT": wqkvT,
            "wprojT": wprojT,
            "gamma_t": gamma_t,
            "beta_t": beta_t,
            "bproj_bc": bproj_b,
        })
    return in_maps


def kernel(x, Wqkv, gamma, beta, Wproj, bproj, **_ignored):
    from concourse.bass_utils import run_bass_kernel_spmd

    nc = _get_nc()
    in_maps = make_in_maps(x, Wqkv, gamma, beta, Wproj, bproj)
    res = run_bass_kernel_spmd(nc, in_maps, core_ids=list(range(8)))
    out = np.empty((B, N, C), np.float32)
    for i in range(8):
        b, h = i // 2, i % 2
        out[b, h * NL:(h + 1) * NL, :] = res.results[i]["out"]
    return out
